# revision 13
# baseline (speedup 1.0000x reference)
"""Trainium2 Bass kernel for nn_CCNNCAModel (RFF + convexified chunk attention).

Contract: kernel(Z, W_rff, A) takes FULL inputs, returns the FULL output tuple
(predictions [N,3], AW [N,3], reg_loss scalar, alpha [256]) matching
reference.py. Internally: pure data-parallel shard of Z rows over 8 NeuronCores;
the only cross-core communication is an AllReduce of the [512] per-column
trig sums that feed the chunk-attention scores.

Algorithm per core (M = N/8 rows, streamed in 512-row tiles):
  phase 1: proj = W^T @ Z^T (PE)  ->  range-reduce (DVE/GPSIMD)  ->
           sin/cos via ACT Sin table with accum_out giving column sums free
  AllReduce [128x4] column sums -> scores -> softmax alpha -> fold alpha
           and the 1/16 Q scaling into A (tiny [512,3] per-row scale)
  phase 2: recompute trig front-end, preds^T[3,512] = sum_t A_s_t^T @ Q_t (PE),
           bounce preds^T to DRAM
  phase 2b: reload preds in [128,3,M/128] layout, Exp/softmax, write
           predictions + AW in natural [M,3] layout.

Sin table on TRN2 is only valid on ~[-3.3, 3.3] (no range reduction), so:
  k = rint(x * inv2pi)   (fp32->int32 convert rounds to nearest)   [DVE]
  m = x - 2*pi*k         (scalar_tensor_tensor, int32*float mixed) [DVE]
  sin(x) = Sin(m);  cos(x) = Sin(-|m| + pi/2), |m| = max(m, -m)    [GPSIMD stt]
"""

import sys

sys.path.insert(0, "/opt/trn_rl_repo")

import numpy as np

import concourse.bass as bass
import concourse.mybir as mybir
import concourse.tile as tile
from concourse import bacc
from concourse.masks import make_identity

N_CORES = 8
D_IN = 64
RFF_DIM = 256
MT = 512  # rows per streamed tile (one PSUM bank of fp32)

f32 = mybir.dt.float32
i32 = mybir.dt.int32
A_OP = mybir.AluOpType
ACT_F = mybir.ActivationFunctionType

TWO_PI = float(2.0 * np.pi)
INV_2PI = float(1.0 / (2.0 * np.pi))
HALF_PI = float(np.pi / 2.0)
MAGIC = float(1.5 * 2**23)
# keeps |scale*m| strictly below pi for the simulator's exact range assert
SIN_SCALE = float(1.0 - 2e-6)


def build_module(m: int, n_cores: int = N_CORES, reduce_mode: str = "i32"):
    """Build the per-core Bass module (SPMD; same program on every core)."""
    ntiles = m // MT
    assert m % MT == 0 and m % 128 == 0
    mm = m // 128  # phase-2b free width per partition

    nc = bacc.Bacc(
        "TRN2", target_bir_lowering=False, debug=False, num_devices=n_cores
    )
    zt_d = nc.dram_tensor("zt", [D_IN, m], f32, kind="ExternalInput")
    w_d = nc.dram_tensor("w", [D_IN, RFF_DIM], f32, kind="ExternalInput")
    a4_d = nc.dram_tensor("a4", [128, 12], f32, kind="ExternalInput")
    c01_d = nc.dram_tensor("c01", [4, 2], f32, kind="ExternalInput")
    preds_d = nc.dram_tensor("preds", [m, 3], f32, kind="ExternalOutput")
    aw_d = nc.dram_tensor("aw", [m, 3], f32, kind="ExternalOutput")
    alpha_d = nc.dram_tensor("alpha", [1, RFF_DIM], f32, kind="ExternalOutput")

    with tile.TileContext(nc) as tc:
        with (
            tc.tile_pool(name="singles", bufs=1) as singles,
            tc.tile_pool(name="mp", bufs=3) as mp,
            tc.tile_pool(name="qp", bufs=3) as qp,
            tc.tile_pool(name="glue", bufs=1) as glue,
            tc.tile_pool(name="pp", bufs=4, space="PSUM") as pp,
            tc.tile_pool(name="ppd", bufs=2, space="PSUM") as ppd,
            tc.tile_pool(name="pglue", bufs=1, space="PSUM") as pglue,
            tc.tile_pool(name="dramp", bufs=1, space="DRAM") as dramp,
        ):
            # ---- persistent loads ----
            zt = singles.tile([D_IN, m], f32)
            ch = min(4096, m)
            for j in range(0, m, ch):
                nc.sync.dma_start(out=zt[:, j : j + ch], in_=zt_d.ap()[:, j : j + ch])
            w = singles.tile([D_IN, RFF_DIM], f32)
            nc.sync.dma_start(out=w, in_=w_d.ap())
            a4 = singles.tile([128, 12], f32)
            nc.sync.dma_start(out=a4, in_=a4_d.ap())
            c01 = singles.tile([4, 2], f32)
            nc.sync.dma_start(out=c01, in_=c01_d.ap())
            halfpi = singles.tile([128, 1], f32)
            nc.vector.memset(halfpi, HALF_PI)
            one1 = singles.tile([1, 1], f32)
            nc.vector.memset(one1, 1.0)
            ident = singles.tile([128, 128], f32)
            make_identity(nc, ident[:])
            accs = singles.tile([128, 4 * ntiles], f32)

            def trig_front(i: int, phase1: bool):
                """proj matmuls + range reduction + sin/cos of row-tile i.

                Returns [cos_lo, cos_hi, sin_lo, sin_hi] tiles ([128, MT]);
                partition p of tile c holds raw trig of flat Q column
                j = 128*c + p (cos for c<2, sin for c>=2)."""
                sl = bass.ds(i * MT, MT)
                out_tiles = [None] * 4
                for h in range(2):
                    pj = pp.tile([128, MT], f32, tag="pj")
                    nc.tensor.matmul(
                        pj,
                        lhsT=w[:, h * 128 : (h + 1) * 128],
                        rhs=zt[:, sl],
                        start=True,
                        stop=True,
                    )
                    mr = mp.tile([128, MT], f32, tag="mr")
                    if reduce_mode == "i32":
                        k32 = mp.tile([128, MT], i32, tag="k32")
                        nc.vector.tensor_scalar_mul(k32, pj, INV_2PI)
                        nc.vector.scalar_tensor_tensor(
                            out=mr, in0=k32, scalar=-TWO_PI, in1=pj,
                            op0=A_OP.mult, op1=A_OP.add,
                        )
                        sin_scale = 1.0
                    else:  # pure-fp32 magic-number rint; bit-exact in the sim
                        t = mp.tile([128, MT], f32, tag="k32")
                        nc.vector.tensor_scalar(
                            out=t, in0=pj, scalar1=INV_2PI, scalar2=MAGIC,
                            op0=A_OP.mult, op1=A_OP.add,
                        )
                        k2pi = mp.tile([128, MT], f32, tag="k2pi")
                        nc.gpsimd.tensor_scalar(
                            out=k2pi, in0=t, scalar1=MAGIC, scalar2=TWO_PI,
                            op0=A_OP.subtract, op1=A_OP.mult,
                        )
                        nc.vector.tensor_sub(mr, pj, k2pi)
                        sin_scale = SIN_SCALE
                    am = mp.tile([128, MT], f32, tag="am")
                    nc.vector.tensor_scalar(
                        out=am.bitcast(i32), in0=mr.bitcast(i32),
                        scalar1=0x7FFFFFFF, scalar2=None, op0=A_OP.bitwise_and,
                    )
                    cos_t = qp.tile([128, MT], f32, tag=f"q{h}c")
                    sin_t = qp.tile([128, MT], f32, tag=f"q{h}s")
                    if phase1:
                        nc.scalar.activation(
                            out=cos_t, in_=am, func=ACT_F.Sin,
                            bias=halfpi[:], scale=-1.0,
                            accum_out=accs[:, h * ntiles + i : h * ntiles + i + 1],
                        )
                        nc.scalar.activation(
                            out=sin_t, in_=mr, func=ACT_F.Sin, scale=sin_scale,
                            accum_out=accs[
                                :, (2 + h) * ntiles + i : (2 + h) * ntiles + i + 1
                            ],
                        )
                    else:
                        nc.scalar.activation(
                            out=cos_t, in_=am, func=ACT_F.Sin,
                            bias=halfpi[:], scale=-1.0,
                        )
                        nc.scalar.activation(
                            out=sin_t, in_=mr, func=ACT_F.Sin, scale=sin_scale
                        )
                    out_tiles[h] = cos_t
                    out_tiles[2 + h] = sin_t
                return out_tiles

            # ---- phase 1: column sums of raw cos/sin ----
            for i in range(ntiles):
                trig_front(i, phase1=True)

            colsum = glue.tile([128, 4], f32)
            for c in range(4):
                nc.vector.reduce_sum(
                    colsum[:, c : c + 1],
                    accs[:, c * ntiles : (c + 1) * ntiles],
                    axis=mybir.AxisListType.X,
                )

            # ---- AllReduce over cores ----
            if n_cores > 1:
                cin = dramp.tile([128, 4], f32)
                cout = dramp.tile([128, 4], f32)
                nc.sync.dma_start(out=cin, in_=colsum)
                nc.gpsimd.collective_compute(
                    "AllReduce",
                    A_OP.add,
                    replica_groups=[list(range(n_cores))],
                    ins=[cin.opt()],
                    outs=[cout.opt()],
                )
                colg = glue.tile([128, 4], f32)
                nc.sync.dma_start(out=colg, in_=cout)
            else:
                colg = colsum

            # ---- scores -> alpha ----
            xps = pglue.tile([4, 128], f32)  # PSUM: colsum transposed
            nc.tensor.transpose(xps, colg, ident[:])
            xsb = glue.tile([4, 128], f32)
            nc.vector.tensor_copy(xsb, xps)
            x2 = xsb.rearrange("c (f two) -> c f two", two=2)
            t0 = glue.tile([4, 64], f32)
            nc.vector.tensor_scalar_mul(t0, x2[:, :, 0], c01[:, 0:1])
            sc = glue.tile([4, 64], f32)
            nc.vector.scalar_tensor_tensor(
                out=sc, in0=x2[:, :, 1], scalar=c01[:, 1:2], in1=t0,
                op0=A_OP.mult, op1=A_OP.add,
            )
            aflat = glue.tile([1, 256], f32)
            nc.sync.dma_start(
                out=aflat.rearrange("o (c f) -> o c f", c=4), in_=sc
            )
            mx = glue.tile([1, 1], f32)
            nc.vector.reduce_max(mx, aflat, axis=mybir.AxisListType.X)
            nmx = glue.tile([1, 1], f32)
            nc.vector.tensor_scalar_mul(nmx, mx, -1.0)
            ev = glue.tile([1, 256], f32)
            se = glue.tile([1, 1], f32)
            nc.scalar.activation(
                out=ev, in_=aflat, func=ACT_F.Exp, bias=nmx[:], scale=1.0,
                accum_out=se,
            )
            rs = glue.tile([1, 1], f32)
            nc.vector.reciprocal(rs, se)
            alph = glue.tile([1, 256], f32)
            nc.vector.tensor_scalar_mul(alph, ev, rs[:, 0:1])
            nc.sync.dma_start(out=alpha_d.ap(), in_=alph)

            # ---- fold alpha into A: a_s[p, t, c] = alpha[(128t+p)//2] * a4 ----
            aexp = glue.tile([1, 512], f32)
            aet = aexp.rearrange("o (f two) -> o two f", two=2)
            nc.vector.tensor_copy(aet[:, 0, :], alph)
            nc.vector.tensor_copy(aet[:, 1, :], alph)
            sps = pglue.tile([128, 4], f32)
            for t in range(4):
                nc.tensor.matmul(
                    sps[:, t : t + 1],
                    lhsT=aexp[:, t * 128 : (t + 1) * 128],
                    rhs=one1[:],
                    start=True,
                    stop=True,
                )
            ssb = glue.tile([128, 4], f32)
            nc.vector.tensor_copy(ssb, sps)
            a_s = glue.tile([128, 12], f32)
            a4v = a4.rearrange("p (t c) -> p t c", t=4)
            asv = a_s.rearrange("p (t c) -> p t c", t=4)
            for t in range(4):
                nc.vector.tensor_scalar_mul(
                    asv[:, t, :], a4v[:, t, :], ssb[:, t : t + 1]
                )

            # ---- phase 2a: predictions^T, bounced to DRAM ----
            pb = dramp.tile([3, m], f32)
            for i in range(ntiles):
                q = trig_front(i, phase1=False)
                pd = ppd.tile([3, MT], f32, tag="pd")
                for t in range(4):
                    nc.tensor.matmul(
                        pd, lhsT=asv[:, t, :], rhs=q[t],
                        start=(t == 0), stop=(t == 3),
                    )
                pt = qp.tile([3, MT], f32, tag="pt")
                nc.vector.tensor_copy(pt, pd)
                nc.sync.dma_start(out=pb[:, bass.ds(i * MT, MT)], in_=pt)

            # ---- phase 2b: Exp/softmax + natural-layout outputs ----
            pbs = glue.tile([128, mm, 3], f32)
            nc.sync.dma_start(
                out=pbs, in_=pb.rearrange("c (a q) -> a q c", a=128)
            )
            nc.sync.dma_start(
                out=preds_d.ap().rearrange("(a q) c -> a (q c)", a=128),
                in_=pbs.rearrange("p q c -> p (q c)"),
            )
            e3 = glue.tile([128, mm, 3], f32)
            nc.scalar.activation(
                out=e3, in_=pbs, func=ACT_F.Exp, scale=1.0 / 16.0
            )
            ssum = glue.tile([128, mm], f32)
            nc.vector.reduce_sum(ssum, e3, axis=mybir.AxisListType.X)
            rr = glue.tile([128, mm], f32)
            nc.vector.reciprocal(rr, ssum)
            aw3 = glue.tile([128, mm, 3], f32)
            for c in range(3):
                nc.vector.tensor_mul(aw3[:, :, c], e3[:, :, c], rr)
            nc.sync.dma_start(
                out=aw_d.ap().rearrange("(a q) c -> a (q c)", a=128),
                in_=aw3.rearrange("p q c -> p (q c)"),
            )

    nc.compile()
    return nc


def make_in_maps(Z, W_rff, A, n_cores: int = N_CORES):
    """Host-side prep: shard/transpose Z, fold constants, replicate W/A."""
    Z = np.ascontiguousarray(np.asarray(Z, dtype=np.float32))
    W_rff = np.ascontiguousarray(np.asarray(W_rff, dtype=np.float32))
    A = np.ascontiguousarray(np.asarray(A, dtype=np.float32))
    n_total = Z.shape[0]
    m = n_total // n_cores

    ZT = np.ascontiguousarray(Z.T)  # [64, N]
    a_div = (A / 16.0).astype(np.float32)  # fold sqrt(1/256) Q scaling into A
    a4 = np.ascontiguousarray(
        a_div.reshape(4, 128, 3).transpose(1, 0, 2).reshape(128, 12)
    )
    a_mean = A.mean(axis=1)
    denom = 16.0 * float(n_total) * (0.1 * np.sqrt(RFF_DIM))
    c01 = np.tile(
        np.array([[a_mean[0] / denom, a_mean[1] / denom]], dtype=np.float32), (4, 1)
    )
    in_maps = []
    for core in range(n_cores):
        in_maps.append(
            {
                "zt": np.ascontiguousarray(ZT[:, core * m : (core + 1) * m]),
                "w": W_rff,
                "a4": a4,
                "c01": c01,
            }
        )
    return in_maps, m


_CACHE = {}


def _get_runner(m: int, n_cores: int = N_CORES, reduce_mode: str = "i32"):
    """Compile the module once and return (run_fn, put_fn).

    run_fn(device_args) -> list of per-core result dicts. Mirrors the
    multi-core branch of bass2jax.run_bass_via_pjrt but caches the jitted
    callable so repeated calls (and timing loops) don't re-lower."""
    key = (m, n_cores, reduce_mode)
    if key in _CACHE:
        return _CACHE[key]

    import jax
    from jax.sharding import Mesh, PartitionSpec, NamedSharding
    from jax.experimental.shard_map import shard_map
    import concourse.mybir as mybir_
    from concourse import bass2jax

    nc = build_module(m, n_cores, reduce_mode)
    bass2jax.install_neuronx_cc_hook()

    partition_name = (
        nc.partition_id_tensor.name if nc.partition_id_tensor else None
    )
    in_names, out_names, out_avals, zero_outs = [], [], [], []
    for alloc in nc.m.functions[0].allocations:
        if not isinstance(alloc, mybir_.MemoryLocationSet):
            continue
        name = alloc.memorylocations[0].name
        if alloc.kind == "ExternalInput":
            if name != partition_name:
                in_names.append(name)
        elif alloc.kind == "ExternalOutput":
            shape = tuple(alloc.tensor_shape)
            dtype = mybir_.dt.np(alloc.dtype)
            out_names.append(name)
            out_avals.append(jax.core.ShapedArray(shape, dtype))
            zero_outs.append(np.zeros(shape, dtype))
    n_params = len(in_names)
    all_in_names = list(in_names) + list(out_names)
    if partition_name is not None:
        all_in_names.append(partition_name)

    def _body(*args):
        operands = list(args)
        if partition_name is not None:
            operands.append(bass2jax.partition_id_tensor())
        outs = bass2jax._bass_exec_p.bind(
            *operands,
            out_avals=tuple(out_avals),
            in_names=tuple(all_in_names),
            out_names=tuple(out_names),
            lowering_input_output_aliases=(),
            sim_require_finite=True,
            sim_require_nnan=True,
            nc=nc,
        )
        return tuple(outs)

    devices = jax.devices()[:n_cores]
    mesh = Mesh(np.asarray(devices), ("core",))
    in_specs = (PartitionSpec("core"),) * (n_params + len(out_names))
    out_specs = (PartitionSpec("core"),) * len(out_names)
    sharded = jax.jit(
        shard_map(_body, mesh=mesh, in_specs=in_specs, out_specs=out_specs,
                  check_rep=False),
        keep_unused=True,
    )
    sharding = NamedSharding(mesh, PartitionSpec("core"))

    def put_fn(in_maps):
        import jax as _jax

        concat_in = [
            np.concatenate([in_maps[c][nm] for c in range(n_cores)], axis=0)
            for nm in in_names
        ]
        concat_zeros = [
            np.zeros((n_cores * z.shape[0], *z.shape[1:]), z.dtype)
            for z in zero_outs
        ]
        return [
            _jax.device_put(a, sharding) for a in concat_in + concat_zeros
        ]

    def run_fn(device_args):
        out_arrs = sharded(*device_args)
        out_arrs = [np.asarray(o) for o in out_arrs]
        return [
            {
                nm: out_arrs[i].reshape(n_cores, *out_avals[i].shape)[c]
                for i, nm in enumerate(out_names)
            }
            for c in range(n_cores)
        ]

    _CACHE[key] = (run_fn, put_fn, sharded)
    return _CACHE[key]


def kernel(Z, W_rff, A):
    Z = np.asarray(Z, dtype=np.float32)
    A_np = np.asarray(A, dtype=np.float32)
    in_maps, m = make_in_maps(Z, W_rff, A_np, N_CORES)
    run_fn, put_fn, _ = _get_runner(m, N_CORES)
    results = run_fn(put_fn(in_maps))
    preds = np.concatenate([r["preds"] for r in results], axis=0)
    aw = np.concatenate([r["aw"] for r in results], axis=0)
    alpha = results[0]["alpha"].reshape(-1)
    reg_loss = np.float32(
        0.01 * np.linalg.svd(A_np, compute_uv=False).astype(np.float32).sum()
    )
    return preds, aw, reg_loss, alpha
